# revision 1
# baseline (speedup 1.0000x reference)
"""GAT layer (nn_GATLayer) on 8 Trainium2 NeuronCores.

Strategy: row-shard the query/node dimension N=4096 across 8 cores
(512 rows each, flash-attention style).  Each core:
  - gets the full x/proj/scores/skip_w (replicated) + its [512, 4096]
    topology slab + its own x rows,
  - computes h = x @ proj_h for all heads (needed for keys),
  - builds a resident bf16 transposed mask  maskT[j, i] (PE transpose),
  - for each head: E^T[j,i] = mask * max(exp(ss_i+sd_j), exp(0.2(ss_i+sd_j)))
    [ = mask * exp(leaky_relu(ss_i + sd_j)) since exp is monotone ]
    computed with ACT (exp, per-partition bias) + fused DVE scalar_tensor_tensor,
  - one matmul per (head, j-tile): psum[65, 512] += h_aug[j,:65]^T @ E^T
    where h_aug's 65th column of ones yields the softmax denominator,
  - normalizes, accumulates heads, adds the (head-summed) skip projection,
    mean over heads + leaky relu, writes its 512 output rows.
No collectives: the host just concatenates the 8 disjoint row blocks.
"""
import os
import numpy as np

PROBE = os.environ.get("KPROBE", "")

N, F_IN, F_OUT, H, NCORES = 4096, 128, 64, 8, 8
NB = N // NCORES          # 512 rows per core
P = 128
NT_J = N // P             # 32 key tiles
NT_I = NB // P            # 4 own-row tiles
TI = NB                   # free-dim width of E tiles (the whole i-block)
NEG = 0.2                 # leaky relu slope

_CACHE = {}


def _build_module(reps=1):
    """Build the bass module.  reps>1 emits the whole kernel body that many
    times (timing aid: device time per rep = slope between rep counts)."""
    import concourse.bacc as bacc
    import concourse.tile as tile
    import concourse.mybir as mybir
    from concourse.masks import make_identity

    dt = mybir.dt
    Alu = mybir.AluOpType
    Act = mybir.ActivationFunctionType

    nc = bacc.Bacc("TRN2", target_bir_lowering=False, debug=False,
                   num_devices=NCORES)

    x_ap = nc.dram_tensor("x", [N, F_IN], dt.float32, kind="ExternalInput").ap()
    xown_ap = nc.dram_tensor("x_own", [NB, F_IN], dt.float32,
                             kind="ExternalInput").ap()
    topo_ap = nc.dram_tensor("topo", [NB, N], dt.float32,
                             kind="ExternalInput").ap()
    proj_ap = nc.dram_tensor("proj", [H, F_IN, F_OUT], dt.float32,
                             kind="ExternalInput").ap()
    ssrc_ap = nc.dram_tensor("score_src", [H, F_OUT], dt.float32,
                             kind="ExternalInput").ap()
    sdst_ap = nc.dram_tensor("score_dst", [H, F_OUT], dt.float32,
                             kind="ExternalInput").ap()
    skw_ap = nc.dram_tensor("skip_w", [H * F_OUT, F_IN], dt.float32,
                            kind="ExternalInput").ap()
    out_ap = nc.dram_tensor("out", [NB, F_OUT], dt.float32,
                            kind="ExternalOutput").ap()

    with tile.TileContext(nc) as tc:
      for _rep in range(reps):
            with (
                tc.tile_pool(name=f"const{_rep}", bufs=1) as cpool,
                tc.tile_pool(name=f"resident{_rep}", bufs=1) as rpool,
                tc.tile_pool(name=f"stage{_rep}", bufs=2) as spool,
                tc.tile_pool(name=f"head{_rep}", bufs=2) as hpool,
                tc.tile_pool(name=f"ew{_rep}", bufs=3) as epool,
                tc.tile_pool(name=f"psum{_rep}", bufs=2, space="PSUM") as ppool,
            ):
                # ---------------- constants -----------------
                id_sb = cpool.tile([P, P], dt.float32)
                make_identity(nc, id_sb[:])
                id_bf = cpool.tile([P, P], dt.bfloat16)
                make_identity(nc, id_bf[:])
                id64 = cpool.tile([64, 64], dt.float32)
                make_identity(nc, id64[:])
                # id2[p, o] = 1 if p % 64 == o  (sums the two head blocks per tile)
                id2 = cpool.tile([P, 64], dt.float32)
                nc.sync.dma_start(id2[0:64, :], id64[:, :])
                nc.sync.dma_start(id2[64:128, :], id64[:, :])

                # ---------------- input staging -----------------
                # x staged in 4 chunks of 8 n-tiles, each [128, 8*128]
                x_chunks = []
                for g in range(4):
                    xc = spool.tile([P, 8 * P], dt.float32, tag="xchunk",
                                    name=f"x_chunk{g}")
                    nc.sync.dma_start(
                        xc.rearrange("p (k f) -> p k f", k=8),
                        x_ap[1024 * g:1024 * (g + 1), :].rearrange(
                            "(k p) f -> p k f", p=P))
                    x_chunks.append(xc)
                xown_sb = spool.tile([P, NT_I * P], dt.float32)
                nc.sync.dma_start(
                    xown_sb.rearrange("p (k f) -> p k f", k=NT_I),
                    xown_ap.rearrange("(k p) f -> p k f", p=P))

                proj_sb = cpool.tile([P, H * F_OUT], dt.float32)
                nc.sync.dma_start(proj_sb.rearrange("p (h o) -> p h o", h=H),
                                  proj_ap.rearrange("h f o -> f h o"))

                sw_sb = []
                for t in range(4):
                    sw_t = spool.tile([P, F_IN], dt.float32, tag="sw",
                                      name=f"sw{t}")
                    nc.sync.dma_start(sw_t[:], skw_ap[t * P:(t + 1) * P, :])
                    sw_sb.append(sw_t)

                # score pads: per q (pair of heads) a [128, 4] tile:
                # cols = [src_{2q}, src_{2q+1}, dst_{2q}, dst_{2q+1}]
                score_pads = []
                for q in range(4):
                    pad = cpool.tile([P, 4], dt.float32, name=f"spad{q}")
                    nc.vector.memset(pad[:], 0.0)
                    nc.sync.dma_start(pad[0:64, 0:1],
                                      ssrc_ap[2 * q:2 * q + 1, :].rearrange(
                                          "a o -> o a"))
                    nc.sync.dma_start(pad[64:128, 1:2],
                                      ssrc_ap[2 * q + 1:2 * q + 2, :].rearrange(
                                          "a o -> o a"))
                    nc.sync.dma_start(pad[0:64, 2:3],
                                      sdst_ap[2 * q:2 * q + 1, :].rearrange(
                                          "a o -> o a"))
                    nc.sync.dma_start(pad[64:128, 3:4],
                                      sdst_ap[2 * q + 1:2 * q + 2, :].rearrange(
                                          "a o -> o a"))
                    score_pads.append(pad)

                # ---------------- xT (transpose of x) -----------------
                xT = rpool.tile([P, N], dt.float32)       # [f, n]
                for nt in range(NT_J):
                    g, k = nt // 8, nt % 8
                    tp = ppool.tile([P, P], dt.float32, tag="tp", name=f"xt{nt}")
                    nc.tensor.transpose(tp[:], x_chunks[g][:, k * P:(k + 1) * P],
                                        id_sb[:])
                    nc.scalar.copy(xT[:, nt * P:(nt + 1) * P], tp[:])
                xTo = rpool.tile([P, NB], dt.float32)     # own rows [f, i]
                for k in range(NT_I):
                    tp = ppool.tile([P, P], dt.float32, tag="tp", name=f"xto{k}")
                    nc.tensor.transpose(tp[:], xown_sb[:, k * P:(k + 1) * P],
                                        id_sb[:])
                    nc.scalar.copy(xTo[:, k * P:(k + 1) * P], tp[:])

                # ---------------- w vectors (proj_h @ score_h) --------------
                # w_all cols per q-block: [w_src_{2q}, w_src_{2q+1},
                #                          w_dst_{2q}, w_dst_{2q+1}]
                w_all = cpool.tile([P, 16], dt.float32)
                for q in range(4):
                    tp = ppool.tile([P, P], dt.float32, tag="tp", name=f"pjt{q}")
                    nc.tensor.transpose(tp[:], proj_sb[:, q * P:(q + 1) * P],
                                        id_sb[:])
                    pjT = spool.tile([P, P], dt.float32, tag="pjT",
                                     name=f"pjTs{q}")
                    nc.scalar.copy(pjT[:], tp[:])
                    wps = ppool.tile([P, 4], dt.float32, tag="small", bufs=1,
                                     name=f"wps{q}")
                    nc.tensor.matmul(wps[:], pjT[:], score_pads[q][:],
                                     start=True, stop=True)
                    nc.scalar.copy(w_all[:, q * 4:(q + 1) * 4], wps[:])

                # ---------------- s vectors -----------------
                # s_sb[:, nt*16 + c]  (c as in w_all cols) for all 4096 nodes
                s_sb = rpool.tile([P, NT_J * 16], dt.float32)
                for nt in range(NT_J):
                    sps = ppool.tile([P, 16], dt.float32, tag="small", bufs=1,
                                     name=f"sps{nt}")
                    nc.tensor.matmul(sps[:], xT[:, nt * P:(nt + 1) * P], w_all[:],
                                     start=True, stop=True)
                    nc.scalar.copy(s_sb[:, nt * 16:(nt + 1) * 16], sps[:])
                # own-row s (query side), transposed to sT [16, 512]
                sT = rpool.tile([16, NB], dt.float32)
                so_sb = spool.tile([P, NT_I * 16], dt.float32)
                for k in range(NT_I):
                    sps = ppool.tile([P, 16], dt.float32, tag="small", bufs=1,
                                     name=f"sop{k}")
                    nc.tensor.matmul(sps[:], xTo[:, k * P:(k + 1) * P], w_all[:],
                                     start=True, stop=True)
                    nc.scalar.copy(so_sb[:, k * 16:(k + 1) * 16], sps[:])
                for k in range(NT_I):
                    tp16 = ppool.tile([16, P], dt.float32, tag="tp",
                                      name=f"st{k}")
                    nc.tensor.transpose(tp16[:], so_sb[:, k * 16:(k + 1) * 16],
                                        id_sb[:])
                    nc.scalar.copy(sT[:, k * P:(k + 1) * P], tp16[:])

                # exp(s_dst) for all nodes + 0.2*s_dst  (per-partition columns)
                # within a 16 block, dst col of head h = 4*(h//2) + 2 + h%2
                sdsel = s_sb.rearrange("p (nt q c) -> p nt q c", q=4, c=4)[
                    :, :, :, 2:4]
                v_sb = rpool.tile([P, NT_J * 8], dt.float32)
                sd2_sb = rpool.tile([P, NT_J * 8], dt.float32)
                nc.scalar.activation(
                    v_sb.rearrange("p (nt q c) -> p nt q c", q=4, c=2),
                    sdsel, Act.Exp)
                nc.vector.tensor_scalar(
                    sd2_sb.rearrange("p (nt q c) -> p nt q c", q=4, c=2),
                    sdsel, 0.2, None, Alu.mult)

                # resident transposed adjacency mask, bf16 {0,1}
                maskT = rpool.tile([P, NT_J * TI], dt.bfloat16)

                # ---------------- topology staging + natural mask ------------
                # stage fp32 topo [128, 2048] halves (bufs=2, recycled fast),
                # convert to a natural-layout bf16 mask immediately (DVE 2x),
                # transposes below read the bf16 mask.
                mask_nat = {}
                for half in range(2):
                    for it in range(NT_I):
                        tt_ = spool.tile([P, N // 2], dt.float32, tag="topo",
                                         bufs=2, name=f"topo{half}_{it}")
                        nc.sync.dma_start(
                            tt_[:], topo_ap[it * P:(it + 1) * P,
                                            half * 2048:(half + 1) * 2048])
                        mn = spool.tile([P, N // 2], dt.bfloat16,
                                        tag="mnat", bufs=4,
                                        name=f"mnat{half}_{it}")
                        nc.vector.tensor_scalar(mn[:], tt_[:], -0.5, None,
                                                Alu.is_ge)
                        mask_nat[(half, it)] = mn

                # ---------------- h_aug (all heads + ones col, bf16) ---------
                h_aug = rpool.tile([P, NT_J * (H * 65)], dt.bfloat16)
                h_aug4 = h_aug.rearrange("p (nt h o) -> p nt h o", h=H, o=65)
                nc.vector.memset(h_aug4[:, :, :, 64:65], 1.0)
                for nt in range(NT_J):
                    hps = ppool.tile([P, H * F_OUT], dt.float32, tag="big",
                                     name=f"hps{nt}")
                    nc.tensor.matmul(hps[:], xT[:, nt * P:(nt + 1) * P],
                                     proj_sb[:], start=True, stop=True)
                    nc.scalar.copy(h_aug4[:, nt, :, 0:64],
                                   hps.rearrange("p (h o) -> p h o", h=H))

                # ---------------- skip path: WsumT = (1/H) sum_h skip_w_h ----
                wsum_ps = ppool.tile([P, 64], dt.float32, tag="small", bufs=1)
                for t in range(4):
                    nc.tensor.matmul(wsum_ps[:], sw_sb[t][:], id2[:],
                                     start=(t == 0), stop=(t == 3))
                wsumT = cpool.tile([P, 64], dt.float32)
                nc.scalar.mul(wsumT[:], wsum_ps[:], 1.0 / H)

                # ---------------- main loop over head pairs -----------------
                acc = [cpool.tile([P, F_OUT], dt.float32, name=f"acc{k}")
                       for k in range(NT_I)]
                for hp in range(H // 2):
                    hds = (2 * hp, 2 * hp + 1)
                    ss_bs, u_bs, ps65 = [], [], []
                    for hd in hds:
                        q, l = hd // 2, hd % 2
                        src_row = 4 * q + l
                        ssrow = hpool.tile([1, TI], dt.float32, tag="ssrow",
                                           name=f"ssrow{hd}")
                        nc.sync.dma_start(ssrow[:], sT[src_row:src_row + 1, :])
                        ss_b = hpool.tile([P, TI], dt.float32, tag=f"ssb{l}",
                                          name=f"ssb{hd}")
                        nc.gpsimd.partition_broadcast(ss_b[:], ssrow[:])
                        ss_bs.append(ss_b)
                        u_b = hpool.tile([P, TI], dt.bfloat16, tag=f"ub{l}",
                                         name=f"ub{hd}")
                        nc.scalar.activation(u_b[:], ss_b[:], Act.Exp)
                        u_bs.append(u_b)
                        ps65.append(ppool.tile([65, 512], dt.float32,
                                               tag="acc65", bufs=3, name=f"ps65_{hd}"))
                    for jc0 in range(0, NT_J, 2):
                        if hp == 0:
                            # build maskT blocks jc0, jc0+1: 8 PE transposes
                            # of the natural bf16 mask + 2 ACT copies
                            for jc in (jc0, jc0 + 1):
                                half, jj = jc // 16, jc % 16
                                tp4 = ppool.tile([P, 4 * P], dt.bfloat16,
                                                 tag="big", name=f"mt{jc}")
                                for it in range(NT_I):
                                    nc.tensor.transpose(
                                        tp4[:, it * P:(it + 1) * P],
                                        mask_nat[(half, it)][
                                            :, jj * P:(jj + 1) * P],
                                        id_bf[:])
                                nc.scalar.copy(
                                    maskT[:, jc * TI:(jc + 1) * TI], tp4[:])
                        # e0q layout: [p, (z, jc2, i)] for 2 heads x 2 j-tiles
                        e0q = epool.tile([P, 4 * TI], dt.bfloat16, tag="e0q", bufs=3,
                                         name=f"e0q_{hp}_{jc0}")
                        for z, hd in enumerate(hds):
                            q, l = hd // 2, hd % 2
                            for jc2 in range(2):
                                jc = jc0 + jc2
                                col = jc * 8 + q * 2 + l
                                dst = e0q[:, (2 * z + jc2) * TI:
                                          (2 * z + jc2 + 1) * TI]
                                if PROBE == "nostt":
                                    nc.scalar.activation(
                                        dst, ss_bs[z][:], Act.Exp,
                                        bias=sd2_sb[:, col:col + 1],
                                        scale=0.2)
                                    continue
                                t2 = epool.tile([P, TI], dt.bfloat16,
                                                tag=f"t2_{z}", bufs=4,
                                                name=f"t2_{hd}_{jc}")
                                nc.scalar.activation(
                                    t2[:], ss_bs[z][:], Act.Exp,
                                    bias=sd2_sb[:, col:col + 1], scale=0.2)
                                nc.vector.scalar_tensor_tensor(
                                    dst, u_bs[z][:], v_sb[:, col:col + 1],
                                    t2[:], Alu.mult, Alu.max)
                        eeq = epool.tile([P, 4 * TI], dt.bfloat16,
                                         tag="eeq", bufs=3, name=f"eeq_{hp}_{jc0}")
                        mb = maskT[:, jc0 * TI:(jc0 + 2) * TI].unsqueeze(
                            1).to_broadcast([P, 2, 2 * TI])
                        nc.vector.tensor_mul(
                            eeq.rearrange("p (z ji) -> p z ji", z=2),
                            e0q.rearrange("p (z ji) -> p z ji", z=2), mb)
                        for z, hd in enumerate(hds):
                            for jc2 in range(2):
                                jc = jc0 + jc2
                                nc.tensor.matmul(
                                    ps65[z][:], h_aug4[:, jc, hd, :],
                                    eeq[:, (2 * z + jc2) * TI:
                                        (2 * z + jc2 + 1) * TI],
                                    start=(jc == 0),
                                    stop=(jc == NT_J - 1))

                    # normalize + accumulate this head pair
                    for z, hd in enumerate(hds):
                        nd_sb = hpool.tile([65, 512], dt.float32, tag=f"nd{z}",
                                           name=f"nd{hd}")
                        nc.scalar.copy(nd_sb[:], ps65[z][:])
                        for it in range(NT_I):
                            tpn = ppool.tile([P, 65], dt.float32, tag="tp",
                                             name=f"ndt{hd}_{it}")
                            nc.tensor.transpose(tpn[:],
                                                nd_sb[:, it * P:(it + 1) * P],
                                                id_sb[0:65, 0:65])
                            rc = hpool.tile([P, 1], dt.float32, tag="rc",
                                            name=f"rc{hd}_{it}")
                            nc.vector.reciprocal(rc[:], tpn[:, 64:65])
                            rc8 = hpool.tile([P, 1], dt.float32, tag="rc8",
                                             name=f"rc8{hd}_{it}")
                            nc.scalar.mul(rc8[:], rc[:], 1.0 / H)
                            if hd == 0:
                                nc.scalar.activation(
                                    acc[it][:], tpn[:, 0:64], Act.Identity,
                                    bias=0.0, scale=rc8[:, 0:1])
                            else:
                                nc.vector.scalar_tensor_tensor(
                                    acc[it][:], tpn[:, 0:64], rc8[:, 0:1],
                                    acc[it][:], Alu.mult, Alu.add)

                # ---------------- skip + mean + leaky relu + out -------------
                for it in range(NT_I):
                    skp = ppool.tile([P, 64], dt.float32, tag="small", bufs=1,
                                     name=f"skp{it}")
                    nc.tensor.matmul(skp[:], xTo[:, it * P:(it + 1) * P],
                                     wsumT[:], start=True, stop=True)
                    qs = hpool.tile([P, F_OUT], dt.float32, tag="qs",
                                    name=f"qs{it}")
                    nc.vector.tensor_add(qs[:], acc[it][:], skp[:])
                    q2 = hpool.tile([P, F_OUT], dt.float32, tag="q2",
                                    name=f"q2_{it}")
                    nc.vector.tensor_scalar_mul(q2[:], qs[:], NEG)
                    ot = hpool.tile([P, F_OUT], dt.float32, tag="ot",
                                    name=f"ot{it}")
                    nc.vector.tensor_max(ot[:], qs[:], q2[:])
                    nc.sync.dma_start(out_ap[it * P:(it + 1) * P, :], ot[:])

    nc.compile()
    return nc


def _get_module(reps=1):
    if reps not in _CACHE:
        _CACHE[reps] = _build_module(reps)
    return _CACHE[reps]


def _make_in_maps(x, topology, proj, score_src, score_dst, skip_w):
    x = np.ascontiguousarray(x, dtype=np.float32)
    topology = np.ascontiguousarray(topology, dtype=np.float32)
    proj = np.ascontiguousarray(proj, dtype=np.float32)
    score_src = np.ascontiguousarray(score_src, dtype=np.float32)
    score_dst = np.ascontiguousarray(score_dst, dtype=np.float32)
    skip_w = np.ascontiguousarray(skip_w, dtype=np.float32)
    in_maps = []
    for c in range(NCORES):
        in_maps.append({
            "x": x,
            "x_own": np.ascontiguousarray(x[c * NB:(c + 1) * NB, :]),
            "topo": np.ascontiguousarray(topology[c * NB:(c + 1) * NB, :]),
            "proj": proj,
            "score_src": score_src,
            "score_dst": score_dst,
            "skip_w": skip_w,
        })
    return in_maps


def _fast_runner(nc):
    """Cached sharded PJRT runner (same _bass_exec_p path that
    run_bass_kernel_spmd uses under axon, but jit-cached across calls)."""
    import jax
    from jax.sharding import Mesh, PartitionSpec
    from jax.experimental.shard_map import shard_map
    from concourse import mybir
    from concourse.bass2jax import (_bass_exec_p, partition_id_tensor,
                                    install_neuronx_cc_hook)

    install_neuronx_cc_hook()
    partition_name = (nc.partition_id_tensor.name
                      if nc.partition_id_tensor else None)
    in_names, out_names, out_avals = [], [], []
    for alloc in nc.m.functions[0].allocations:
        if not isinstance(alloc, mybir.MemoryLocationSet):
            continue
        name = alloc.memorylocations[0].name
        if alloc.kind == "ExternalInput":
            if name != partition_name:
                in_names.append(name)
        elif alloc.kind == "ExternalOutput":
            out_names.append(name)
            out_avals.append(jax.core.ShapedArray(
                tuple(alloc.tensor_shape), mybir.dt.np(alloc.dtype)))
    n_params = len(in_names)
    all_in = list(in_names) + list(out_names)
    if partition_name is not None:
        all_in.append(partition_name)

    def _body(*args):
        operands = list(args)
        if partition_name is not None:
            operands.append(partition_id_tensor())
        return tuple(_bass_exec_p.bind(
            *operands, out_avals=tuple(out_avals), in_names=tuple(all_in),
            out_names=tuple(out_names), lowering_input_output_aliases=(),
            sim_require_finite=True, sim_require_nnan=True, nc=nc))

    devices = jax.devices()[:NCORES]
    mesh = Mesh(np.asarray(devices), ("core",))
    specs_in = (PartitionSpec("core"),) * (n_params + len(out_names))
    specs_out = (PartitionSpec("core"),) * len(out_names)
    fn = jax.jit(shard_map(_body, mesh=mesh, in_specs=specs_in,
                           out_specs=specs_out, check_rep=False),
                 keep_unused=True)
    zero_shapes = [(NCORES * a.shape[0], *a.shape[1:]) for a in out_avals]
    zero_dtypes = [a.dtype for a in out_avals]

    def run(in_maps):
        per_core = [[np.asarray(m[nm]) for nm in in_names] for m in in_maps]
        cin = [np.concatenate([per_core[c][i] for c in range(NCORES)], axis=0)
               for i in range(n_params)]
        cz = [np.zeros(s, d) for s, d in zip(zero_shapes, zero_dtypes)]
        outs = fn(*cin, *cz)
        o = np.asarray(outs[out_names.index("out")])
        return o.reshape(NCORES, NB, F_OUT)

    return run


def kernel(x, topology, proj, score_src, score_dst, skip_w):
    nc = _get_module()
    in_maps = _make_in_maps(x, topology, proj, score_src, score_dst, skip_w)
    if "runner" not in _CACHE:
        # first call: go through the canonical entry point
        from concourse.bass_utils import run_bass_kernel_spmd
        res = run_bass_kernel_spmd(nc, in_maps, list(range(NCORES)))
        _CACHE["runner"] = _fast_runner(nc)
        out = np.concatenate([res.results[c]["out"] for c in range(NCORES)],
                             axis=0)
        return out.astype(np.float32)
    per_core = _CACHE["runner"](in_maps)
    return np.concatenate(list(per_core), axis=0).astype(np.float32)



# revision 6
# speedup vs baseline: 6.2559x; 6.2559x over previous
"""GAT layer (nn_GATLayer) on 8 Trainium2 NeuronCores.

Strategy: row-shard the query/node dimension N=4096 across 8 cores
(512 rows each, flash-attention style).  Each core:
  - gets the full x/proj/scores/skip_w (replicated) + its [512, 4096]
    topology slab + its own x rows,
  - computes h = x @ proj_h for all heads (needed for keys),
  - builds a resident bf16 transposed mask  maskT[j, i] (ACT relu(topo+1)
    for the {0,1} mask + PE transpose),
  - for each head, softmax rows can be rescaled freely, so with
    u=exp(ss), v=exp(sd), q=exp(0.2 sd), w=exp(-0.8 ss):
      E^T[j,i] = mask * exp(leaky_relu(ss_i+sd_j)) / u_i
               = mask * max(v_j, w_i * q_j)
    i.e. ONE dve tensor_scalar (mult+max, 4x perf mode: all the per-j
    factors are [P,1] scalar operands) + ONE 2x masked multiply.  No
    per-element ACT exp / 1x scalar_tensor_tensor at all,
  - one matmul per (head, j-tile): psum[65, 512] += h_aug[j,:65]^T @ E^T
    where h_aug's 65th column of ones yields the softmax denominator,
  - normalizes, accumulates heads, adds the (head-summed) skip projection,
    mean over heads + leaky relu, writes its 512 output rows.
No collectives: the host just concatenates the 8 disjoint row blocks.
"""
import numpy as np

N, F_IN, F_OUT, H, NCORES = 4096, 128, 64, 8, 8
NB = N // NCORES          # 512 rows per core
P = 128
NT_J = N // P             # 32 key tiles
NT_I = NB // P            # 4 own-row tiles
TI = NB                   # free-dim width of E tiles (the whole i-block)
NEG = 0.2                 # leaky relu slope

_CACHE = {}


def _build_module(reps=1):
    """Build the bass module.  reps>1 emits the whole kernel body that many
    times (timing aid: device time per rep = slope between rep counts)."""
    import concourse.bacc as bacc
    import concourse.tile as tile
    import concourse.mybir as mybir
    from concourse.masks import make_identity

    dt = mybir.dt
    Alu = mybir.AluOpType
    Act = mybir.ActivationFunctionType

    nc = bacc.Bacc("TRN2", target_bir_lowering=False, debug=False,
                   num_devices=NCORES)

    x_ap = nc.dram_tensor("x", [N, F_IN], dt.float32, kind="ExternalInput").ap()
    xown_ap = nc.dram_tensor("x_own", [NB, F_IN], dt.float32,
                             kind="ExternalInput").ap()
    topo_ap = nc.dram_tensor("topo", [NB, N], dt.float32,
                             kind="ExternalInput").ap()
    proj_ap = nc.dram_tensor("proj", [H, F_IN, F_OUT], dt.float32,
                             kind="ExternalInput").ap()
    ssrc_ap = nc.dram_tensor("score_src", [H, F_OUT], dt.float32,
                             kind="ExternalInput").ap()
    sdst_ap = nc.dram_tensor("score_dst", [H, F_OUT], dt.float32,
                             kind="ExternalInput").ap()
    skw_ap = nc.dram_tensor("skip_w", [H * F_OUT, F_IN], dt.float32,
                            kind="ExternalInput").ap()
    out_ap = nc.dram_tensor("out", [NB, F_OUT], dt.float32,
                            kind="ExternalOutput").ap()

    with tile.TileContext(nc) as tc:
      for _rep in range(reps):
            with (
                tc.tile_pool(name=f"const{_rep}", bufs=1) as cpool,
                tc.tile_pool(name=f"resident{_rep}", bufs=1) as rpool,
                tc.tile_pool(name=f"stage{_rep}", bufs=2) as spool,
                tc.tile_pool(name=f"head{_rep}", bufs=2) as hpool,
                tc.tile_pool(name=f"ew{_rep}", bufs=3) as epool,
                tc.tile_pool(name=f"psum{_rep}", bufs=2, space="PSUM") as ppool,
            ):
                # ---------------- constants -----------------
                id_sb = cpool.tile([P, P], dt.float32)
                make_identity(nc, id_sb[:])
                id_bf = cpool.tile([P, P], dt.bfloat16)
                make_identity(nc, id_bf[:])
                id64 = cpool.tile([64, 64], dt.float32)
                make_identity(nc, id64[:])
                # id2[p, o] = 1 if p % 64 == o  (sums the two head blocks per tile)
                id2 = cpool.tile([P, 64], dt.float32)
                nc.sync.dma_start(id2[0:64, :], id64[:, :])
                nc.sync.dma_start(id2[64:128, :], id64[:, :])

                # ---------------- input staging -----------------
                # x staged in 4 chunks of 8 n-tiles, each [128, 8*128]
                x_chunks = []
                for g in range(4):
                    xc = spool.tile([P, 8 * P], dt.float32, tag="xchunk",
                                    name=f"x_chunk{g}")
                    nc.sync.dma_start(
                        xc.rearrange("p (k f) -> p k f", k=8),
                        x_ap[1024 * g:1024 * (g + 1), :].rearrange(
                            "(k p) f -> p k f", p=P))
                    x_chunks.append(xc)
                xown_sb = spool.tile([P, NT_I * P], dt.float32)
                nc.sync.dma_start(
                    xown_sb.rearrange("p (k f) -> p k f", k=NT_I),
                    xown_ap.rearrange("(k p) f -> p k f", p=P))

                proj_sb = cpool.tile([P, H * F_OUT], dt.float32)
                nc.sync.dma_start(proj_sb.rearrange("p (h o) -> p h o", h=H),
                                  proj_ap.rearrange("h f o -> f h o"))

                sw_sb = []
                for t in range(4):
                    sw_t = spool.tile([P, F_IN], dt.float32, tag="sw",
                                      name=f"sw{t}")
                    nc.sync.dma_start(sw_t[:], skw_ap[t * P:(t + 1) * P, :])
                    sw_sb.append(sw_t)

                # score pads: per q (pair of heads) a [128, 4] tile:
                # cols = [src_{2q}, src_{2q+1}, dst_{2q}, dst_{2q+1}]
                score_pads = []
                for q in range(4):
                    pad = cpool.tile([P, 4], dt.float32, name=f"spad{q}")
                    nc.vector.memset(pad[:], 0.0)
                    nc.sync.dma_start(pad[0:64, 0:1],
                                      ssrc_ap[2 * q:2 * q + 1, :].rearrange(
                                          "a o -> o a"))
                    nc.sync.dma_start(pad[64:128, 1:2],
                                      ssrc_ap[2 * q + 1:2 * q + 2, :].rearrange(
                                          "a o -> o a"))
                    nc.sync.dma_start(pad[0:64, 2:3],
                                      sdst_ap[2 * q:2 * q + 1, :].rearrange(
                                          "a o -> o a"))
                    nc.sync.dma_start(pad[64:128, 3:4],
                                      sdst_ap[2 * q + 1:2 * q + 2, :].rearrange(
                                          "a o -> o a"))
                    score_pads.append(pad)

                # ---------------- xT (transpose of x) -----------------
                xT = rpool.tile([P, N], dt.float32)       # [f, n]
                for nt in range(NT_J):
                    g, k = nt // 8, nt % 8
                    tp = ppool.tile([P, P], dt.float32, tag="tp", name=f"xt{nt}")
                    nc.tensor.transpose(tp[:], x_chunks[g][:, k * P:(k + 1) * P],
                                        id_sb[:])
                    nc.scalar.copy(xT[:, nt * P:(nt + 1) * P], tp[:])
                xTo = rpool.tile([P, NB], dt.float32)     # own rows [f, i]
                for k in range(NT_I):
                    tp = ppool.tile([P, P], dt.float32, tag="tp", name=f"xto{k}")
                    nc.tensor.transpose(tp[:], xown_sb[:, k * P:(k + 1) * P],
                                        id_sb[:])
                    nc.scalar.copy(xTo[:, k * P:(k + 1) * P], tp[:])

                # ---------------- w vectors (proj_h @ score_h) --------------
                # w_all cols per q-block: [w_src_{2q}, w_src_{2q+1},
                #                          w_dst_{2q}, w_dst_{2q+1}]
                w_all = cpool.tile([P, 16], dt.float32)
                for q in range(4):
                    tp = ppool.tile([P, P], dt.float32, tag="tp", name=f"pjt{q}")
                    nc.tensor.transpose(tp[:], proj_sb[:, q * P:(q + 1) * P],
                                        id_sb[:])
                    pjT = spool.tile([P, P], dt.float32, tag="pjT",
                                     name=f"pjTs{q}")
                    nc.scalar.copy(pjT[:], tp[:])
                    wps = ppool.tile([P, 4], dt.float32, tag="small", bufs=1,
                                     name=f"wps{q}")
                    nc.tensor.matmul(wps[:], pjT[:], score_pads[q][:],
                                     start=True, stop=True)
                    nc.scalar.copy(w_all[:, q * 4:(q + 1) * 4], wps[:])

                # ---------------- s vectors -----------------
                # s_sb[:, nt*16 + c]  (c as in w_all cols) for all 4096 nodes
                s_sb = rpool.tile([P, NT_J * 16], dt.float32)
                for nt in range(NT_J):
                    sps = ppool.tile([P, 16], dt.float32, tag="small", bufs=1,
                                     name=f"sps{nt}")
                    nc.tensor.matmul(sps[:], xT[:, nt * P:(nt + 1) * P], w_all[:],
                                     start=True, stop=True)
                    nc.scalar.copy(s_sb[:, nt * 16:(nt + 1) * 16], sps[:])
                # own-row s (query side), transposed to sT [16, 512]
                sT = rpool.tile([16, NB], dt.float32)
                so_sb = spool.tile([P, NT_I * 16], dt.float32)
                for k in range(NT_I):
                    sps = ppool.tile([P, 16], dt.float32, tag="small", bufs=1,
                                     name=f"sop{k}")
                    nc.tensor.matmul(sps[:], xTo[:, k * P:(k + 1) * P], w_all[:],
                                     start=True, stop=True)
                    nc.scalar.copy(so_sb[:, k * 16:(k + 1) * 16], sps[:])
                for k in range(NT_I):
                    tp16 = ppool.tile([16, P], dt.float32, tag="tp",
                                      name=f"st{k}")
                    nc.tensor.transpose(tp16[:], so_sb[:, k * 16:(k + 1) * 16],
                                        id_sb[:])
                    nc.scalar.copy(sT[:, k * P:(k + 1) * P], tp16[:])

                # per-partition key-side columns, head h = 2q+l at col nt*8+h:
                #   v = exp(s_dst), q = exp(0.2 * s_dst)
                # within a 16 block, dst col of head h = 4*(h//2) + 2 + h%2
                sdsel = s_sb.rearrange("p (nt q c) -> p nt q c", q=4, c=4)[
                    :, :, :, 2:4]
                v_sb = rpool.tile([P, NT_J * 8], dt.float32)
                q_sb = rpool.tile([P, NT_J * 8], dt.float32)
                nc.scalar.activation(
                    v_sb.rearrange("p (nt q c) -> p nt q c", q=4, c=2),
                    sdsel, Act.Exp)
                nc.scalar.activation(
                    q_sb.rearrange("p (nt q c) -> p nt q c", q=4, c=2),
                    sdsel, Act.Exp, scale=0.2)

                # resident transposed adjacency mask, bf16 {0,1}
                maskT = rpool.tile([P, NT_J * TI], dt.bfloat16)

                # ---------------- topology staging + natural mask ------------
                # stage fp32 topo [128, 2048] halves (bufs=2, recycled fast),
                # convert to a natural-layout bf16 mask immediately (DVE 2x),
                # transposes below read the bf16 mask.
                mask_nat = {}
                for half in range(2):
                    for it in range(NT_I):
                        tt_ = spool.tile([P, N // 2], dt.float32, tag="topo",
                                         bufs=2, name=f"topo{half}_{it}")
                        nc.sync.dma_start(
                            tt_[:], topo_ap[it * P:(it + 1) * P,
                                            half * 2048:(half + 1) * 2048])
                        mn = spool.tile([P, N // 2], dt.bfloat16,
                                        tag="mnat", bufs=4,
                                        name=f"mnat{half}_{it}")
                        # topo is 0 (edge) or -1e9: relu(topo + 1) = {1, 0}
                        # on the (otherwise idle) ACT engine.
                        nc.scalar.activation(mn[:], tt_[:], Act.Relu, bias=1.0)
                        mask_nat[(half, it)] = mn

                # ---------------- h_aug (all heads + ones col, bf16) ---------
                h_aug = rpool.tile([P, NT_J * (H * 65)], dt.bfloat16)
                h_aug4 = h_aug.rearrange("p (nt h o) -> p nt h o", h=H, o=65)
                nc.vector.memset(h_aug4[:, :, :, 64:65], 1.0)
                for nt in range(NT_J):
                    hps = ppool.tile([P, H * F_OUT], dt.float32, tag="big",
                                     name=f"hps{nt}")
                    nc.tensor.matmul(hps[:], xT[:, nt * P:(nt + 1) * P],
                                     proj_sb[:], start=True, stop=True)
                    nc.scalar.copy(h_aug4[:, nt, :, 0:64],
                                   hps.rearrange("p (h o) -> p h o", h=H))

                # ---------------- skip path: WsumT = (1/H) sum_h skip_w_h ----
                wsum_ps = ppool.tile([P, 64], dt.float32, tag="small", bufs=1)
                for t in range(4):
                    nc.tensor.matmul(wsum_ps[:], sw_sb[t][:], id2[:],
                                     start=(t == 0), stop=(t == 3))
                wsumT = cpool.tile([P, 64], dt.float32)
                nc.scalar.mul(wsumT[:], wsum_ps[:], 1.0 / H)

                # ---------------- main loop over head pairs -----------------
                acc = [cpool.tile([P, F_OUT], dt.float32, name=f"acc{k}")
                       for k in range(NT_I)]
                for hp in range(H // 2):
                    hds = (2 * hp, 2 * hp + 1)
                    w_bs, ps65 = [], []
                    for hd in hds:
                        q, l = hd // 2, hd % 2
                        src_row = 4 * q + l
                        ssrow = hpool.tile([1, TI], dt.float32, tag="ssrow",
                                           name=f"ssrow{hd}")
                        nc.sync.dma_start(ssrow[:], sT[src_row:src_row + 1, :])
                        ss_b = hpool.tile([P, TI], dt.float32, tag=f"ssb{l}",
                                          name=f"ssb{hd}")
                        nc.gpsimd.partition_broadcast(ss_b[:], ssrow[:])
                        # w_i = exp(-0.8 * ss_i), broadcast along partitions
                        w_b = hpool.tile([P, TI], dt.bfloat16, tag=f"ub{l}",
                                         name=f"wb{hd}")
                        nc.scalar.activation(w_b[:], ss_b[:], Act.Exp,
                                             scale=-0.8)
                        w_bs.append(w_b)
                        ps65.append(ppool.tile([65, 512], dt.float32,
                                               tag="acc65", bufs=3, name=f"ps65_{hd}"))
                    for jc0 in range(0, NT_J, 2):
                        if hp == 0:
                            # build maskT blocks jc0, jc0+1: 8 PE transposes
                            # of the natural bf16 mask + 2 ACT copies
                            for jc in (jc0, jc0 + 1):
                                half, jj = jc // 16, jc % 16
                                tp4 = ppool.tile([P, 4 * P], dt.bfloat16,
                                                 tag="big", name=f"mt{jc}")
                                for it in range(NT_I):
                                    nc.tensor.transpose(
                                        tp4[:, it * P:(it + 1) * P],
                                        mask_nat[(half, it)][
                                            :, jj * P:(jj + 1) * P],
                                        id_bf[:])
                                nc.scalar.copy(
                                    maskT[:, jc * TI:(jc + 1) * TI], tp4[:])
                        # e0q layout: [p, (z, jc2, i)] for 2 heads x 2 j-tiles
                        # e0q[j, i] = max(w_i * q_j, v_j): one 4x-mode dve
                        # tensor_scalar per (head, j-tile).
                        e0q = epool.tile([P, 4 * TI], dt.bfloat16, tag="e0q", bufs=3,
                                         name=f"e0q_{hp}_{jc0}")
                        for z, hd in enumerate(hds):
                            q, l = hd // 2, hd % 2
                            for jc2 in range(2):
                                jc = jc0 + jc2
                                col = jc * 8 + q * 2 + l
                                dst = e0q[:, (2 * z + jc2) * TI:
                                          (2 * z + jc2 + 1) * TI]
                                nc.vector.tensor_scalar(
                                    dst, w_bs[z][:], q_sb[:, col:col + 1],
                                    v_sb[:, col:col + 1], Alu.mult, Alu.max)
                        eeq = epool.tile([P, 4 * TI], dt.bfloat16,
                                         tag="eeq", bufs=3, name=f"eeq_{hp}_{jc0}")
                        mb = maskT[:, jc0 * TI:(jc0 + 2) * TI].unsqueeze(
                            1).to_broadcast([P, 2, 2 * TI])
                        nc.vector.tensor_mul(
                            eeq.rearrange("p (z ji) -> p z ji", z=2),
                            e0q.rearrange("p (z ji) -> p z ji", z=2), mb)
                        for z, hd in enumerate(hds):
                            for jc2 in range(2):
                                jc = jc0 + jc2
                                nc.tensor.matmul(
                                    ps65[z][:], h_aug4[:, jc, hd, :],
                                    eeq[:, (2 * z + jc2) * TI:
                                        (2 * z + jc2 + 1) * TI],
                                    start=(jc == 0),
                                    stop=(jc == NT_J - 1))

                    # normalize + accumulate this head pair
                    for z, hd in enumerate(hds):
                        nd_sb = hpool.tile([65, 512], dt.float32, tag=f"nd{z}",
                                           name=f"nd{hd}")
                        nc.scalar.copy(nd_sb[:], ps65[z][:])
                        for it in range(NT_I):
                            tpn = ppool.tile([P, 65], dt.float32, tag="tp",
                                             name=f"ndt{hd}_{it}")
                            nc.tensor.transpose(tpn[:],
                                                nd_sb[:, it * P:(it + 1) * P],
                                                id_sb[0:65, 0:65])
                            rc = hpool.tile([P, 1], dt.float32, tag="rc",
                                            name=f"rc{hd}_{it}")
                            nc.vector.reciprocal(rc[:], tpn[:, 64:65])
                            rc8 = hpool.tile([P, 1], dt.float32, tag="rc8",
                                             name=f"rc8{hd}_{it}")
                            nc.scalar.mul(rc8[:], rc[:], 1.0 / H)
                            if hd == 0:
                                nc.scalar.activation(
                                    acc[it][:], tpn[:, 0:64], Act.Identity,
                                    bias=0.0, scale=rc8[:, 0:1])
                            else:
                                nc.vector.scalar_tensor_tensor(
                                    acc[it][:], tpn[:, 0:64], rc8[:, 0:1],
                                    acc[it][:], Alu.mult, Alu.add)

                # ---------------- skip + mean + leaky relu + out -------------
                for it in range(NT_I):
                    skp = ppool.tile([P, 64], dt.float32, tag="small", bufs=1,
                                     name=f"skp{it}")
                    nc.tensor.matmul(skp[:], xTo[:, it * P:(it + 1) * P],
                                     wsumT[:], start=True, stop=True)
                    qs = hpool.tile([P, F_OUT], dt.float32, tag="qs",
                                    name=f"qs{it}")
                    nc.vector.tensor_add(qs[:], acc[it][:], skp[:])
                    q2 = hpool.tile([P, F_OUT], dt.float32, tag="q2",
                                    name=f"q2_{it}")
                    nc.vector.tensor_scalar_mul(q2[:], qs[:], NEG)
                    ot = hpool.tile([P, F_OUT], dt.float32, tag="ot",
                                    name=f"ot{it}")
                    nc.vector.tensor_max(ot[:], qs[:], q2[:])
                    nc.sync.dma_start(out_ap[it * P:(it + 1) * P, :], ot[:])

    nc.compile()
    return nc


def _get_module(reps=1):
    if reps not in _CACHE:
        _CACHE[reps] = _build_module(reps)
    return _CACHE[reps]


def _make_in_maps(x, topology, proj, score_src, score_dst, skip_w):
    x = np.ascontiguousarray(x, dtype=np.float32)
    topology = np.ascontiguousarray(topology, dtype=np.float32)
    proj = np.ascontiguousarray(proj, dtype=np.float32)
    score_src = np.ascontiguousarray(score_src, dtype=np.float32)
    score_dst = np.ascontiguousarray(score_dst, dtype=np.float32)
    skip_w = np.ascontiguousarray(skip_w, dtype=np.float32)
    in_maps = []
    for c in range(NCORES):
        in_maps.append({
            "x": x,
            "x_own": np.ascontiguousarray(x[c * NB:(c + 1) * NB, :]),
            "topo": np.ascontiguousarray(topology[c * NB:(c + 1) * NB, :]),
            "proj": proj,
            "score_src": score_src,
            "score_dst": score_dst,
            "skip_w": skip_w,
        })
    return in_maps


def _fast_runner(nc):
    """Cached sharded PJRT runner (same _bass_exec_p path that
    run_bass_kernel_spmd uses under axon, but jit-cached across calls)."""
    import jax
    from jax.sharding import Mesh, PartitionSpec
    from jax.experimental.shard_map import shard_map
    from concourse import mybir
    from concourse.bass2jax import (_bass_exec_p, partition_id_tensor,
                                    install_neuronx_cc_hook)

    install_neuronx_cc_hook()
    partition_name = (nc.partition_id_tensor.name
                      if nc.partition_id_tensor else None)
    in_names, out_names, out_avals = [], [], []
    for alloc in nc.m.functions[0].allocations:
        if not isinstance(alloc, mybir.MemoryLocationSet):
            continue
        name = alloc.memorylocations[0].name
        if alloc.kind == "ExternalInput":
            if name != partition_name:
                in_names.append(name)
        elif alloc.kind == "ExternalOutput":
            out_names.append(name)
            out_avals.append(jax.core.ShapedArray(
                tuple(alloc.tensor_shape), mybir.dt.np(alloc.dtype)))
    n_params = len(in_names)
    all_in = list(in_names) + list(out_names)
    if partition_name is not None:
        all_in.append(partition_name)

    def _body(*args):
        operands = list(args)
        if partition_name is not None:
            operands.append(partition_id_tensor())
        return tuple(_bass_exec_p.bind(
            *operands, out_avals=tuple(out_avals), in_names=tuple(all_in),
            out_names=tuple(out_names), lowering_input_output_aliases=(),
            sim_require_finite=True, sim_require_nnan=True, nc=nc))

    devices = jax.devices()[:NCORES]
    mesh = Mesh(np.asarray(devices), ("core",))
    specs_in = (PartitionSpec("core"),) * (n_params + len(out_names))
    specs_out = (PartitionSpec("core"),) * len(out_names)
    fn = jax.jit(shard_map(_body, mesh=mesh, in_specs=specs_in,
                           out_specs=specs_out, check_rep=False),
                 keep_unused=True)
    zero_shapes = [(NCORES * a.shape[0], *a.shape[1:]) for a in out_avals]
    zero_dtypes = [a.dtype for a in out_avals]

    def run(in_maps):
        per_core = [[np.asarray(m[nm]) for nm in in_names] for m in in_maps]
        cin = [np.concatenate([per_core[c][i] for c in range(NCORES)], axis=0)
               for i in range(n_params)]
        cz = [np.zeros(s, d) for s, d in zip(zero_shapes, zero_dtypes)]
        outs = fn(*cin, *cz)
        o = np.asarray(outs[out_names.index("out")])
        return o.reshape(NCORES, NB, F_OUT)

    return run


def kernel(x, topology, proj, score_src, score_dst, skip_w):
    nc = _get_module()
    in_maps = _make_in_maps(x, topology, proj, score_src, score_dst, skip_w)
    if "runner" not in _CACHE:
        # first call: go through the canonical entry point
        from concourse.bass_utils import run_bass_kernel_spmd
        res = run_bass_kernel_spmd(nc, in_maps, list(range(NCORES)))
        _CACHE["runner"] = _fast_runner(nc)
        out = np.concatenate([res.results[c]["out"] for c in range(NCORES)],
                             axis=0)
        return out.astype(np.float32)
    per_core = _CACHE["runner"](in_maps)
    return np.concatenate(list(per_core), axis=0).astype(np.float32)



# revision 13
# speedup vs baseline: 9.0851x; 1.4522x over previous
"""GAT layer (nn_GATLayer) on 8 Trainium2 NeuronCores.

Strategy: row-shard the query/node dimension N=4096 across 8 cores
(512 rows each, flash-attention style).  Each core:
  - gets the full x/proj/scores/skip_w (replicated) + its [512, 4096]
    topology slab + its own x rows,
  - computes h = x @ proj_h for all heads (needed for keys),
  - builds a resident bf16 transposed mask  maskT[j, i] (ACT relu(topo+1)
    for the {0,1} mask + PE transpose),
  - for each head, softmax rows can be rescaled freely, so with
    u=exp(ss), v=exp(sd), q=exp(0.2 sd), w=exp(-0.8 ss):
      E^T[j,i] = mask * exp(leaky_relu(ss_i+sd_j)) / u_i
               = mask * max(v_j, w_i * q_j)
    i.e. ONE dve tensor_scalar (mult+max, 4x perf mode: all the per-j
    factors are [P,1] scalar operands) + ONE 2x masked multiply.  No
    per-element ACT exp / 1x scalar_tensor_tensor at all,
  - one matmul per (head, j-tile): psum[65, 512] += h_aug[j,:65]^T @ E^T
    where h_aug's 65th column of ones yields the softmax denominator,
  - normalizes, accumulates heads, adds the (head-summed) skip projection,
    mean over heads + leaky relu, writes its 512 output rows.
No collectives: the host just concatenates the 8 disjoint row blocks.
"""
import os
import numpy as np

PROBE = os.environ.get("KPROBE", "")

N, F_IN, F_OUT, H, NCORES = 4096, 128, 64, 8, 8
NB = N // NCORES          # 512 rows per core
P = 128
NT_J = N // P             # 32 key tiles
NT_I = NB // P            # 4 own-row tiles
TI = NB                   # free-dim width of E tiles (the whole i-block)
NEG = 0.2                 # leaky relu slope

_CACHE = {}


def _build_module(reps=1):
    """Build the bass module.  reps>1 emits the whole kernel body that many
    times (timing aid: device time per rep = slope between rep counts)."""
    import concourse.bacc as bacc
    import concourse.tile as tile
    import concourse.mybir as mybir
    from concourse.masks import make_identity

    dt = mybir.dt
    Alu = mybir.AluOpType
    Act = mybir.ActivationFunctionType

    nc = bacc.Bacc("TRN2", target_bir_lowering=False, debug=False,
                   num_devices=NCORES)

    x_ap = nc.dram_tensor("x", [N, F_IN], dt.float32, kind="ExternalInput").ap()
    xown_ap = nc.dram_tensor("x_own", [NB, F_IN], dt.float32,
                             kind="ExternalInput").ap()
    topo_ap = nc.dram_tensor("topo", [NB, N], dt.float32,
                             kind="ExternalInput").ap()
    proj_ap = nc.dram_tensor("proj", [H, F_IN, F_OUT], dt.float32,
                             kind="ExternalInput").ap()
    ssrc_ap = nc.dram_tensor("score_src", [H, F_OUT], dt.float32,
                             kind="ExternalInput").ap()
    sdst_ap = nc.dram_tensor("score_dst", [H, F_OUT], dt.float32,
                             kind="ExternalInput").ap()
    skw_ap = nc.dram_tensor("skip_w", [H * F_OUT, F_IN], dt.float32,
                            kind="ExternalInput").ap()
    out_ap = nc.dram_tensor("out", [NB, F_OUT], dt.float32,
                            kind="ExternalOutput").ap()

    with tile.TileContext(nc) as tc:
      for _rep in range(reps):
            with (
                tc.tile_pool(name=f"const{_rep}", bufs=1) as cpool,
                tc.tile_pool(name=f"resident{_rep}", bufs=1) as rpool,
                tc.tile_pool(name=f"stage{_rep}", bufs=2) as spool,
                tc.tile_pool(name=f"head{_rep}", bufs=2) as hpool,
                tc.tile_pool(name=f"ew{_rep}", bufs=3) as epool,
                tc.tile_pool(name=f"psum{_rep}", bufs=2, space="PSUM") as ppool,
            ):
                # ---------------- constants -----------------
                id_sb = cpool.tile([P, P], dt.float32)
                make_identity(nc, id_sb[:])
                id_bf = cpool.tile([P, P], dt.bfloat16)
                make_identity(nc, id_bf[:])
                id64 = cpool.tile([64, 64], dt.float32)
                make_identity(nc, id64[:])
                # id2[p, o] = 1 if p % 64 == o  (sums the two head blocks per tile)
                id2 = cpool.tile([P, 64], dt.float32)
                nc.sync.dma_start(id2[0:64, :], id64[:, :])
                nc.sync.dma_start(id2[64:128, :], id64[:, :])

                # ---------------- input staging -----------------
                # x staged in 4 chunks of 8 n-tiles, each [128, 8*128]
                x_chunks = []
                for g in range(4):
                    xc = spool.tile([P, 8 * P], dt.float32, tag="xchunk",
                                    name=f"x_chunk{g}")
                    nc.sync.dma_start(
                        xc.rearrange("p (k f) -> p k f", k=8),
                        x_ap[1024 * g:1024 * (g + 1), :].rearrange(
                            "(k p) f -> p k f", p=P))
                    x_chunks.append(xc)
                xown_sb = spool.tile([P, NT_I * P], dt.float32)
                nc.sync.dma_start(
                    xown_sb.rearrange("p (k f) -> p k f", k=NT_I),
                    xown_ap.rearrange("(k p) f -> p k f", p=P))

                proj_sb = cpool.tile([P, H * F_OUT], dt.float32)
                nc.sync.dma_start(proj_sb.rearrange("p (h o) -> p h o", h=H),
                                  proj_ap.rearrange("h f o -> f h o"))

                sw_sb = []
                for t in range(4):
                    sw_t = spool.tile([P, F_IN], dt.float32, tag="sw",
                                      name=f"sw{t}")
                    nc.sync.dma_start(sw_t[:], skw_ap[t * P:(t + 1) * P, :])
                    sw_sb.append(sw_t)

                # score pads: per q (pair of heads) a [128, 4] tile:
                # cols = [src_{2q}, src_{2q+1}, dst_{2q}, dst_{2q+1}]
                score_pads = []
                for q in range(4):
                    pad = cpool.tile([P, 4], dt.float32, name=f"spad{q}")
                    nc.vector.memset(pad[:], 0.0)
                    nc.sync.dma_start(pad[0:64, 0:1],
                                      ssrc_ap[2 * q:2 * q + 1, :].rearrange(
                                          "a o -> o a"))
                    nc.sync.dma_start(pad[64:128, 1:2],
                                      ssrc_ap[2 * q + 1:2 * q + 2, :].rearrange(
                                          "a o -> o a"))
                    nc.sync.dma_start(pad[0:64, 2:3],
                                      sdst_ap[2 * q:2 * q + 1, :].rearrange(
                                          "a o -> o a"))
                    nc.sync.dma_start(pad[64:128, 3:4],
                                      sdst_ap[2 * q + 1:2 * q + 2, :].rearrange(
                                          "a o -> o a"))
                    score_pads.append(pad)

                # ---------------- xT (transpose of x) -----------------
                # float32r tiles: the ACT evac copy performs the f32r
                # rounding the BIR verifier wants, and f32r matmuls run at
                # 1 cycle/row (vs 4 for plain fp32) when free >= 256.
                xT = rpool.tile([P, N], dt.float32r)      # [f, n]
                for nt in range(NT_J):
                    g, k = nt // 8, nt % 8
                    tp = ppool.tile([P, P], dt.float32, tag="tp", name=f"xt{nt}")
                    nc.tensor.transpose(tp[:], x_chunks[g][:, k * P:(k + 1) * P],
                                        id_sb[:])
                    nc.scalar.copy(xT[:, nt * P:(nt + 1) * P], tp[:])
                xTo = rpool.tile([P, NB], dt.float32r)    # own rows [f, i]
                for k in range(NT_I):
                    tp = ppool.tile([P, P], dt.float32, tag="tp", name=f"xto{k}")
                    nc.tensor.transpose(tp[:], xown_sb[:, k * P:(k + 1) * P],
                                        id_sb[:])
                    nc.scalar.copy(xTo[:, k * P:(k + 1) * P], tp[:])

                # ---------------- w vectors (proj_h @ score_h) --------------
                # w_all cols per q-block: [w_src_{2q}, w_src_{2q+1},
                #                          w_dst_{2q}, w_dst_{2q+1}]
                w_all = cpool.tile([P, 16], dt.float32r)
                for q in range(4):
                    tp = ppool.tile([P, P], dt.float32, tag="tp", name=f"pjt{q}")
                    nc.tensor.transpose(tp[:], proj_sb[:, q * P:(q + 1) * P],
                                        id_sb[:])
                    pjT = spool.tile([P, P], dt.float32, tag="pjT",
                                     name=f"pjTs{q}")
                    nc.scalar.copy(pjT[:], tp[:])
                    wps = ppool.tile([P, 4], dt.float32, tag="small", bufs=1,
                                     name=f"wps{q}")
                    nc.tensor.matmul(wps[:], pjT[:], score_pads[q][:],
                                     start=True, stop=True)
                    nc.scalar.copy(w_all[:, q * 4:(q + 1) * 4], wps[:])

                # ---------------- s vectors -----------------
                # s_sb[:, nt*16 + c]  (c as in w_all cols) for all 4096 nodes
                s_sb = rpool.tile([P, NT_J * 16], dt.float32)
                for nt in range(NT_J):
                    sps = ppool.tile([P, 16], dt.float32, tag="small", bufs=1,
                                     name=f"sps{nt}")
                    nc.tensor.matmul(sps[:], xT[:, nt * P:(nt + 1) * P], w_all[:],
                                     start=True, stop=True)
                    nc.scalar.copy(s_sb[:, nt * 16:(nt + 1) * 16], sps[:])
                # own-row s (query side), transposed to sT [16, 512]
                sT = rpool.tile([16, NB], dt.float32)
                so_sb = spool.tile([P, NT_I * 16], dt.float32)
                for k in range(NT_I):
                    sps = ppool.tile([P, 16], dt.float32, tag="small", bufs=1,
                                     name=f"sop{k}")
                    nc.tensor.matmul(sps[:], xTo[:, k * P:(k + 1) * P], w_all[:],
                                     start=True, stop=True)
                    nc.scalar.copy(so_sb[:, k * 16:(k + 1) * 16], sps[:])
                for k in range(NT_I):
                    tp16 = ppool.tile([16, P], dt.float32, tag="tp",
                                      name=f"st{k}")
                    nc.tensor.transpose(tp16[:], so_sb[:, k * 16:(k + 1) * 16],
                                        id_sb[:])
                    nc.scalar.copy(sT[:, k * P:(k + 1) * P], tp16[:])

                # per-partition key-side columns, head h = 2q+l at col nt*8+h:
                #   v = exp(s_dst), q = exp(0.2 * s_dst)
                # within a 16 block, dst col of head h = 4*(h//2) + 2 + h%2
                sdsel = s_sb.rearrange("p (nt q c) -> p nt q c", q=4, c=4)[
                    :, :, :, 2:4]
                v_sb = rpool.tile([P, NT_J * 8], dt.float32)
                q_sb = rpool.tile([P, NT_J * 8], dt.float32)
                nc.scalar.activation(
                    v_sb.rearrange("p (nt q c) -> p nt q c", q=4, c=2),
                    sdsel, Act.Exp)
                nc.scalar.activation(
                    q_sb.rearrange("p (nt q c) -> p nt q c", q=4, c=2),
                    sdsel, Act.Exp, scale=0.2)

                # resident transposed adjacency mask, bf16 {0,1}
                maskT = rpool.tile([P, NT_J * TI], dt.bfloat16)

                # ---------------- topology staging + natural mask ------------
                # stage fp32 topo [128, 2048] halves (bufs=2, recycled fast),
                # convert to a natural-layout bf16 mask immediately (DVE 2x),
                # transposes below read the bf16 mask.
                mask_nat = {}
                for half in range(2):
                    for it in range(NT_I):
                        tt_ = spool.tile([P, N // 2], dt.float32, tag="topo",
                                         bufs=2, name=f"topo{half}_{it}")
                        nc.sync.dma_start(
                            tt_[:], topo_ap[it * P:(it + 1) * P,
                                            half * 2048:(half + 1) * 2048])
                        mn = spool.tile([P, N // 2], dt.bfloat16,
                                        tag="mnat", bufs=4,
                                        name=f"mnat{half}_{it}")
                        # topo is 0 (edge) or -1e9: relu(topo + 1) = {1, 0}
                        # on the (otherwise idle) ACT engine.
                        nc.scalar.activation(mn[:], tt_[:], Act.Relu, bias=1.0)
                        mask_nat[(half, it)] = mn

                # ---------------- h_aug (all heads + ones col, bf16) ---------
                h_aug = rpool.tile([P, NT_J * (H * 65)], dt.bfloat16)
                h_aug4 = h_aug.rearrange("p (nt h o) -> p nt h o", h=H, o=65)
                nc.vector.memset(h_aug4[:, :, :, 64:65], 1.0)
                proj_r = cpool.tile([P, H * F_OUT], dt.float32r)
                nc.scalar.copy(proj_r[:], proj_sb[:])
                for nt in range(NT_J):
                    hps = ppool.tile([P, H * F_OUT], dt.float32, tag="big",
                                     name=f"hps{nt}")
                    nc.tensor.matmul(hps[:], xT[:, nt * P:(nt + 1) * P],
                                     proj_r[:], start=True, stop=True)
                    nc.scalar.copy(h_aug4[:, nt, :, 0:64],
                                   hps.rearrange("p (h o) -> p h o", h=H))

                # ---------------- skip path: WsumT = (1/H) sum_h skip_w_h ----
                wsum_ps = ppool.tile([P, 64], dt.float32, tag="small", bufs=1)
                for t in range(4):
                    nc.tensor.matmul(wsum_ps[:], sw_sb[t][:], id2[:],
                                     start=(t == 0), stop=(t == 3))
                wsumT = cpool.tile([P, 64], dt.float32r)
                nc.scalar.mul(wsumT[:], wsum_ps[:], 1.0 / H)

                # ---------------- main loop over head pairs -----------------
                acc = [cpool.tile([P, F_OUT], dt.float32, name=f"acc{k}")
                       for k in range(NT_I)]
                for hp in range(H // 2):
                    hds = (2 * hp, 2 * hp + 1)
                    w_bs, ps65 = [], []
                    for hd in hds:
                        q, l = hd // 2, hd % 2
                        src_row = 4 * q + l
                        ssrow = hpool.tile([1, TI], dt.float32, tag="ssrow",
                                           name=f"ssrow{hd}")
                        nc.sync.dma_start(ssrow[:], sT[src_row:src_row + 1, :])
                        ss_b = hpool.tile([P, TI], dt.float32, tag=f"ssb{l}",
                                          name=f"ssb{hd}")
                        nc.gpsimd.partition_broadcast(ss_b[:], ssrow[:])
                        # w_i = exp(-0.8 * ss_i), broadcast along partitions
                        w_b = hpool.tile([P, TI], dt.bfloat16, tag=f"ub{l}",
                                         name=f"wb{hd}")
                        nc.scalar.activation(w_b[:], ss_b[:], Act.Exp,
                                             scale=-0.8)
                        w_bs.append(w_b)
                        ps65.append(ppool.tile([65, 512], dt.float32,
                                               tag="acc65", bufs=3, name=f"ps65_{hd}"))
                    for jc0 in range(0, NT_J, 2):
                        if hp == 0:
                            # build maskT blocks jc0, jc0+1: 8 PE transposes
                            # of the natural bf16 mask + 2 ACT copies
                            for jc in (jc0, jc0 + 1):
                                half, jj = jc // 16, jc % 16
                                tp4 = ppool.tile([P, 4 * P], dt.bfloat16,
                                                 tag="big", name=f"mt{jc}")
                                for it in range(NT_I):
                                    nc.tensor.transpose(
                                        tp4[:, it * P:(it + 1) * P],
                                        mask_nat[(half, it)][
                                            :, jj * P:(jj + 1) * P],
                                        id_bf[:])
                                nc.scalar.copy(
                                    maskT[:, jc * TI:(jc + 1) * TI], tp4[:])
                        # e0q layout: [p, (z, jc2, i)] for 2 heads x 2 j-tiles
                        # e0q[j, i] = max(w_i * q_j, v_j): one 4x-mode dve
                        # tensor_scalar per (head, j-tile).
                        e0q = epool.tile([P, 4 * TI], dt.bfloat16, tag="e0q", bufs=3,
                                         name=f"e0q_{hp}_{jc0}")
                        if PROBE != "nots":
                            for z, hd in enumerate(hds):
                                q, l = hd // 2, hd % 2
                                for jc2 in range(2):
                                    jc = jc0 + jc2
                                    col = jc * 8 + q * 2 + l
                                    dst = e0q[:, (2 * z + jc2) * TI:
                                              (2 * z + jc2 + 1) * TI]
                                    nc.vector.tensor_scalar(
                                        dst, w_bs[z][:], q_sb[:, col:col + 1],
                                        v_sb[:, col:col + 1], Alu.mult,
                                        Alu.max)
                        eeq = epool.tile([P, 4 * TI], dt.bfloat16,
                                         tag="eeq", bufs=3, name=f"eeq_{hp}_{jc0}")
                        mb = maskT[:, jc0 * TI:(jc0 + 2) * TI].unsqueeze(
                            1).to_broadcast([P, 2, 2 * TI])
                        if PROBE != "nott":
                            nc.vector.tensor_mul(
                                eeq.rearrange("p (z ji) -> p z ji", z=2),
                                e0q.rearrange("p (z ji) -> p z ji", z=2), mb)
                        src_eq = e0q if PROBE == "nott" else eeq
                        if PROBE != "nomm":
                            for z, hd in enumerate(hds):
                                for jc2 in range(2):
                                    jc = jc0 + jc2
                                    nc.tensor.matmul(
                                        ps65[z][:], h_aug4[:, jc, hd, :],
                                        src_eq[:, (2 * z + jc2) * TI:
                                               (2 * z + jc2 + 1) * TI],
                                        start=(jc == 0),
                                        stop=(jc == NT_J - 1))

                    # normalize + accumulate this head pair
                    for z, hd in enumerate(hds):
                        nd_sb = hpool.tile([65, 512], dt.float32, tag=f"nd{z}",
                                           name=f"nd{hd}")
                        nc.scalar.copy(nd_sb[:], ps65[z][:])
                        for it in range(NT_I):
                            tpn = ppool.tile([P, 65], dt.float32, tag="tp",
                                             name=f"ndt{hd}_{it}")
                            nc.tensor.transpose(tpn[:],
                                                nd_sb[:, it * P:(it + 1) * P],
                                                id_sb[0:65, 0:65])
                            rc = hpool.tile([P, 1], dt.float32, tag="rc",
                                            name=f"rc{hd}_{it}")
                            nc.vector.reciprocal(rc[:], tpn[:, 64:65])
                            rc8 = hpool.tile([P, 1], dt.float32, tag="rc8",
                                             name=f"rc8{hd}_{it}")
                            nc.scalar.mul(rc8[:], rc[:], 1.0 / H)
                            if hd == 0:
                                nc.scalar.activation(
                                    acc[it][:], tpn[:, 0:64], Act.Identity,
                                    bias=0.0, scale=rc8[:, 0:1])
                            else:
                                nc.vector.scalar_tensor_tensor(
                                    acc[it][:], tpn[:, 0:64], rc8[:, 0:1],
                                    acc[it][:], Alu.mult, Alu.add)

                # ---------------- skip + mean + leaky relu + out -------------
                for it in range(NT_I):
                    skp = ppool.tile([P, 64], dt.float32, tag="small", bufs=1,
                                     name=f"skp{it}")
                    nc.tensor.matmul(skp[:], xTo[:, it * P:(it + 1) * P],
                                     wsumT[:], start=True, stop=True)
                    qs = hpool.tile([P, F_OUT], dt.float32, tag="qs",
                                    name=f"qs{it}")
                    nc.vector.tensor_add(qs[:], acc[it][:], skp[:])
                    q2 = hpool.tile([P, F_OUT], dt.float32, tag="q2",
                                    name=f"q2_{it}")
                    nc.vector.tensor_scalar_mul(q2[:], qs[:], NEG)
                    ot = hpool.tile([P, F_OUT], dt.float32, tag="ot",
                                    name=f"ot{it}")
                    nc.vector.tensor_max(ot[:], qs[:], q2[:])
                    nc.sync.dma_start(out_ap[it * P:(it + 1) * P, :], ot[:])

    nc.compile()
    return nc


def _get_module(reps=1):
    if reps not in _CACHE:
        _CACHE[reps] = _build_module(reps)
    return _CACHE[reps]


def _make_in_maps(x, topology, proj, score_src, score_dst, skip_w):
    x = np.ascontiguousarray(x, dtype=np.float32)
    topology = np.ascontiguousarray(topology, dtype=np.float32)
    proj = np.ascontiguousarray(proj, dtype=np.float32)
    score_src = np.ascontiguousarray(score_src, dtype=np.float32)
    score_dst = np.ascontiguousarray(score_dst, dtype=np.float32)
    skip_w = np.ascontiguousarray(skip_w, dtype=np.float32)
    in_maps = []
    for c in range(NCORES):
        in_maps.append({
            "x": x,
            "x_own": np.ascontiguousarray(x[c * NB:(c + 1) * NB, :]),
            "topo": np.ascontiguousarray(topology[c * NB:(c + 1) * NB, :]),
            "proj": proj,
            "score_src": score_src,
            "score_dst": score_dst,
            "skip_w": skip_w,
        })
    return in_maps


def _fast_runner(nc):
    """Cached sharded PJRT runner (same _bass_exec_p path that
    run_bass_kernel_spmd uses under axon, but jit-cached across calls)."""
    import jax
    from jax.sharding import Mesh, PartitionSpec
    from jax.experimental.shard_map import shard_map
    from concourse import mybir
    from concourse.bass2jax import (_bass_exec_p, partition_id_tensor,
                                    install_neuronx_cc_hook)

    install_neuronx_cc_hook()
    partition_name = (nc.partition_id_tensor.name
                      if nc.partition_id_tensor else None)
    in_names, out_names, out_avals = [], [], []
    for alloc in nc.m.functions[0].allocations:
        if not isinstance(alloc, mybir.MemoryLocationSet):
            continue
        name = alloc.memorylocations[0].name
        if alloc.kind == "ExternalInput":
            if name != partition_name:
                in_names.append(name)
        elif alloc.kind == "ExternalOutput":
            out_names.append(name)
            out_avals.append(jax.core.ShapedArray(
                tuple(alloc.tensor_shape), mybir.dt.np(alloc.dtype)))
    n_params = len(in_names)
    all_in = list(in_names) + list(out_names)
    if partition_name is not None:
        all_in.append(partition_name)

    def _body(*args):
        operands = list(args)
        if partition_name is not None:
            operands.append(partition_id_tensor())
        return tuple(_bass_exec_p.bind(
            *operands, out_avals=tuple(out_avals), in_names=tuple(all_in),
            out_names=tuple(out_names), lowering_input_output_aliases=(),
            sim_require_finite=True, sim_require_nnan=True, nc=nc))

    devices = jax.devices()[:NCORES]
    mesh = Mesh(np.asarray(devices), ("core",))
    specs_in = (PartitionSpec("core"),) * (n_params + len(out_names))
    specs_out = (PartitionSpec("core"),) * len(out_names)
    fn = jax.jit(shard_map(_body, mesh=mesh, in_specs=specs_in,
                           out_specs=specs_out, check_rep=False),
                 keep_unused=True)
    zero_shapes = [(NCORES * a.shape[0], *a.shape[1:]) for a in out_avals]
    zero_dtypes = [a.dtype for a in out_avals]

    def run(in_maps):
        per_core = [[np.asarray(m[nm]) for nm in in_names] for m in in_maps]
        cin = [np.concatenate([per_core[c][i] for c in range(NCORES)], axis=0)
               for i in range(n_params)]
        cz = [np.zeros(s, d) for s, d in zip(zero_shapes, zero_dtypes)]
        outs = fn(*cin, *cz)
        o = np.asarray(outs[out_names.index("out")])
        return o.reshape(NCORES, NB, F_OUT)

    return run


def kernel(x, topology, proj, score_src, score_dst, skip_w):
    nc = _get_module()
    in_maps = _make_in_maps(x, topology, proj, score_src, score_dst, skip_w)
    if "runner" not in _CACHE:
        # first call: go through the canonical entry point
        from concourse.bass_utils import run_bass_kernel_spmd
        res = run_bass_kernel_spmd(nc, in_maps, list(range(NCORES)))
        _CACHE["runner"] = _fast_runner(nc)
        out = np.concatenate([res.results[c]["out"] for c in range(NCORES)],
                             axis=0)
        return out.astype(np.float32)
    per_core = _CACHE["runner"](in_maps)
    return np.concatenate(list(per_core), axis=0).astype(np.float32)

